# revision 2
# baseline (speedup 1.0000x reference)
"""Biquad lowpass filter (torchaudio lowpass_biquad, SR=24000, cutoff=8000, Q=0.707)
over wav [64, 480000], data-parallel across 8 TRN2 NeuronCores.

The biquad's poles have |z| = sqrt(a2) ~= 0.49, so its impulse response decays
below float32 resolution within ~48 samples. The IIR is therefore numerically
identical to a 64-tap causal FIR, which we evaluate on the TensorEngine:

  y[n*128 + i] = sum_k x[n*128 + k] * HB[k, i]  +  sum_k x[(n-1)*128 + k] * HA[k, i]
  HB[k, i] = h[i - k]        (0 <= i-k < D)
  HA[k, i] = h[128 + i - k]  (0 < 128+i-k < D)

Per core: 8 rows x 480000 samples, viewed as 120 chunks (8 rows x 15 chunks)
of 32000 samples. Time slices of 128 are PE-transposed so that within-slice
time sits on partitions, the two banded matmuls run in float32r, and the
result slices are PE-transposed back and streamed out contiguously.
"""

import sys

sys.path.insert(0, "/opt/trn_rl_repo")

import numpy as np

import concourse.bass as bass
import concourse.mybir as mybir
import concourse.tile as tile
from concourse import bacc
from concourse.bass_utils import run_bass_kernel_spmd

f32 = mybir.dt.float32
f32r = mybir.dt.float32r

# ---- problem constants ----------------------------------------------------
SR = 24000
CUTOFF = 8000.0
Q = 0.707

B_FULL, T = 64, 480000
N_CORES = 8
R = B_FULL // N_CORES          # rows per core
NCH = 15                       # chunks per row
P = R * NCH                    # 120 partitions in use
L = T // NCH                   # 32000 samples per chunk
SL = 128                       # slice length (transpose tile)
NSL = L // SL                  # 250 slices per chunk
S = 4                          # slices per pipeline iteration
D = 64                         # FIR taps kept (h[48] ~ 1e-15 already)

# matmul dtype: float32r runs the PE at 1 cycle/column for free-dim >= 256
MM_DT = f32
TP_DT = f32                    # dtype for PE transposes


def _filter_mats():
    w0 = 2.0 * np.pi * CUTOFF / SR
    alpha = np.sin(w0) / (2.0 * Q)
    cos_w0 = np.cos(w0)
    b0 = (1.0 - cos_w0) / 2.0
    b1 = 1.0 - cos_w0
    b2 = b0
    a0 = 1.0 + alpha
    a1 = -2.0 * cos_w0
    a2 = 1.0 - alpha
    b0, b1, b2, a1, a2 = (np.float32(b0 / a0), np.float32(b1 / a0),
                          np.float32(b2 / a0), np.float32(a1 / a0),
                          np.float32(a2 / a0))
    # impulse response in float64 using the float32-rounded coefficients
    h = np.zeros(D, dtype=np.float64)
    x1 = x2 = y1 = y2 = 0.0
    for t in range(D):
        x = 1.0 if t == 0 else 0.0
        y = (float(b0) * x + float(b1) * x1 + float(b2) * x2
             - float(a1) * y1 - float(a2) * y2)
        h[t] = y
        x2, x1 = x1, x
        y2, y1 = y1, y
    HB = np.zeros((SL, SL), dtype=np.float32)
    HA = np.zeros((SL, SL), dtype=np.float32)
    for k in range(SL):
        for i in range(SL):
            d = i - k
            if 0 <= d < D:
                HB[k, i] = h[d]
            d2 = SL + i - k
            if 0 < d2 < D:
                HA[k, i] = h[d2]
    return HA, HB


def _build():
    HA_np, HB_np = _filter_mats()
    nc = bacc.Bacc("TRN2", target_bir_lowering=False)

    wav = nc.dram_tensor("wav", [R, T], f32, kind="ExternalInput")
    out = nc.dram_tensor("out", [R, T], f32, kind="ExternalOutput")
    hA_d = nc.inline_tensor(HA_np, name="hA")
    hB_d = nc.inline_tensor(HB_np, name="hB")
    id_d = nc.inline_tensor(np.eye(SL, dtype=np.float32), name="ident")

    wav_ch = wav[:, :].rearrange("r (c l) -> (r c) l", c=NCH)   # [120, 32000]
    out_ch = out[:, :].rearrange("r (c l) -> (r c) l", c=NCH)

    n_full = (NSL // S) if (NSL % S) else (NSL // S - 1)
    iters = []
    done = 0
    while done < NSL:
        s = min(S, NSL - done)
        iters.append((done, s))
        done += s

    with tile.TileContext(nc) as tc:
        with (
            tc.tile_pool(name="const", bufs=1) as cpool,
            tc.tile_pool(name="io", bufs=3) as iopool,
            tc.tile_pool(name="work", bufs=3) as wpool,
            tc.tile_pool(name="psum", bufs=2, space="PSUM") as ppool,
        ):
            hA = cpool.tile([SL, SL], f32)
            nc.sync.dma_start(hA[:], hA_d[:, :])
            hB = cpool.tile([SL, SL], f32)
            nc.sync.dma_start(hB[:], hB_d[:, :])
            ident = cpool.tile([SL, SL], f32)
            nc.sync.dma_start(ident[:], id_d[:, :])

            # initial carry: the 128 samples preceding each chunk (zeros for
            # row-initial chunks), in X_nat layout then transposed.
            c0 = cpool.tile([P, SL], f32)
            nc.gpsimd.memset(c0[:], 0.0)
            for r in range(R):
                nc.sync.dma_start(
                    c0[r * NCH + 1: r * NCH + NCH, :],
                    wav_ch[r * NCH: r * NCH + NCH - 1, L - SL: L],
                )
            pc0 = ppool.tile([SL, P], f32, tag="pt")
            nc.tensor.transpose(
                pc0[:].bitcast(TP_DT), c0[:].bitcast(TP_DT),
                ident[:P, :P].bitcast(TP_DT),
            )

            prev_slab = None
            prev_s = None
            for (sl0, s) in iters:
                base = sl0 * SL
                w = s * SL

                xin = iopool.tile([P, S * SL], f32, tag="xin")
                nc.sync.dma_start(xin[:, :w], wav_ch[:, base: base + w])

                # transpose s slices of [P, 128] -> [128, P] into one PSUM bank
                pt = ppool.tile([SL, S * P], f32, tag="pt")
                for j in range(s):
                    nc.tensor.transpose(
                        pt[:, j * P: (j + 1) * P].bitcast(TP_DT),
                        xin[:, j * SL: (j + 1) * SL].bitcast(TP_DT),
                        ident[:P, :P].bitcast(TP_DT),
                    )

                # slab: slot 0 = carry (transposed previous slice), 1..s = new
                slab = wpool.tile([SL, (S + 1) * P], f32, tag="slab")
                nc.scalar.copy(slab[:, P: P + s * P], pt[:, : s * P])
                if prev_slab is None:
                    nc.scalar.copy(slab[:, 0: P], pc0[:, :])
                else:
                    nc.vector.tensor_copy(
                        slab[:, 0: P],
                        prev_slab[:, prev_s * P: (prev_s + 1) * P],
                    )

                # banded FIR: y_T = HB^T @ slab[1..s] + HA^T @ slab[0..s-1]
                py = ppool.tile([SL, S * P], f32, tag="py")
                nc.tensor.matmul(
                    py[:, : s * P],
                    hB[:].bitcast(MM_DT),
                    slab[:, P: P + s * P].bitcast(MM_DT),
                    start=True, stop=False,
                )
                nc.tensor.matmul(
                    py[:, : s * P],
                    hA[:].bitcast(MM_DT),
                    slab[:, 0: s * P].bitcast(MM_DT),
                    start=False, stop=True,
                )

                ysb = wpool.tile([SL, S * P], f32, tag="ysb")
                nc.vector.tensor_copy(ysb[:, : s * P], py[:, : s * P])

                # transpose back: [128, P] slices -> [P, 128]
                po = ppool.tile([P, S * SL], f32, tag="po")
                for j in range(s):
                    nc.tensor.transpose(
                        po[:, j * SL: (j + 1) * SL].bitcast(TP_DT),
                        ysb[:, j * P: (j + 1) * P].bitcast(TP_DT),
                        ident[:, :].bitcast(TP_DT),
                    )

                yout = iopool.tile([P, S * SL], f32, tag="yout")
                nc.scalar.copy(yout[:, :w], po[:, :w])
                nc.sync.dma_start(out_ch[:, base: base + w], yout[:, :w])

                prev_slab = slab
                prev_s = s

    nc.finalize()
    return nc


_NC_CACHE = None


def _get_nc():
    global _NC_CACHE
    if _NC_CACHE is None:
        _NC_CACHE = _build()
    return _NC_CACHE


def _run(wav_full: np.ndarray, trace: bool = False):
    nc = _get_nc()
    wav_full = np.ascontiguousarray(wav_full, dtype=np.float32)
    in_maps = [
        {"wav": wav_full[i * R: (i + 1) * R]} for i in range(N_CORES)
    ]
    res = run_bass_kernel_spmd(
        nc, in_maps, core_ids=list(range(N_CORES)), trace=trace
    )
    out = np.concatenate([res.results[i]["out"] for i in range(N_CORES)], axis=0)
    return out, res


def kernel(wav: np.ndarray) -> np.ndarray:
    out, _ = _run(np.asarray(wav))
    return out


# revision 3
# speedup vs baseline: 1.1836x; 1.1836x over previous
"""Biquad lowpass filter (torchaudio lowpass_biquad, SR=24000, cutoff=8000, Q=0.707)
over wav [64, 480000], data-parallel across 8 TRN2 NeuronCores.

The biquad's poles have |z| = sqrt(a2) ~= 0.49, so its impulse response decays
below float32 resolution within ~48 samples. The IIR is therefore numerically
identical to a 64-tap causal FIR, which we evaluate on the TensorEngine:

  y[n*128 + i] = sum_k x[n*128 + k] * HB[k, i]  +  sum_k x[(n-1)*128 + k] * HA[k, i]
  HB[k, i] = h[i - k]        (0 <= i-k < D)
  HA[k, i] = h[128 + i - k]  (0 < 128+i-k < D)

Per core: 8 rows x 480000 samples, viewed as 120 chunks (8 rows x 15 chunks)
of 32000 samples. Time slices of 128 are PE-transposed so that within-slice
time sits on partitions, the two banded matmuls run in float32r, and the
result slices are PE-transposed back and streamed out contiguously.
"""

import sys

sys.path.insert(0, "/opt/trn_rl_repo")

import numpy as np

import concourse.bass as bass
import concourse.mybir as mybir
import concourse.tile as tile
from concourse import bacc
from concourse.bass_utils import run_bass_kernel_spmd

f32 = mybir.dt.float32
f32r = mybir.dt.float32r

# ---- problem constants ----------------------------------------------------
SR = 24000
CUTOFF = 8000.0
Q = 0.707

B_FULL, T = 64, 480000
N_CORES = 8
R = B_FULL // N_CORES          # rows per core
NCH = 15                       # chunks per row
P = R * NCH                    # 120 partitions in use
L = T // NCH                   # 32000 samples per chunk
SL = 128                       # slice length (transpose tile)
NSL = L // SL                  # 250 slices per chunk
S = 4                          # slices per pipeline iteration
D = 64                         # FIR taps kept (h[48] ~ 1e-15 already)

# matmul dtype: float32r runs the PE at 1 cycle/column for free-dim >= 256
MM_DT = f32r
TP_DT = f32                    # dtype for PE transposes


def _filter_mats():
    w0 = 2.0 * np.pi * CUTOFF / SR
    alpha = np.sin(w0) / (2.0 * Q)
    cos_w0 = np.cos(w0)
    b0 = (1.0 - cos_w0) / 2.0
    b1 = 1.0 - cos_w0
    b2 = b0
    a0 = 1.0 + alpha
    a1 = -2.0 * cos_w0
    a2 = 1.0 - alpha
    b0, b1, b2, a1, a2 = (np.float32(b0 / a0), np.float32(b1 / a0),
                          np.float32(b2 / a0), np.float32(a1 / a0),
                          np.float32(a2 / a0))
    # impulse response in float64 using the float32-rounded coefficients
    h = np.zeros(D, dtype=np.float64)
    x1 = x2 = y1 = y2 = 0.0
    for t in range(D):
        x = 1.0 if t == 0 else 0.0
        y = (float(b0) * x + float(b1) * x1 + float(b2) * x2
             - float(a1) * y1 - float(a2) * y2)
        h[t] = y
        x2, x1 = x1, x
        y2, y1 = y1, y
    HB = np.zeros((SL, SL), dtype=np.float32)
    HA = np.zeros((SL, SL), dtype=np.float32)
    for k in range(SL):
        for i in range(SL):
            d = i - k
            if 0 <= d < D:
                HB[k, i] = h[d]
            d2 = SL + i - k
            if 0 < d2 < D:
                HA[k, i] = h[d2]
    return HA, HB


def _build():
    HA_np, HB_np = _filter_mats()
    nc = bacc.Bacc("TRN2", target_bir_lowering=False)

    wav = nc.dram_tensor("wav", [R, T], f32, kind="ExternalInput")
    out = nc.dram_tensor("out", [R, T], f32, kind="ExternalOutput")
    hA_d = nc.inline_tensor(HA_np, name="hA")
    hB_d = nc.inline_tensor(HB_np, name="hB")
    id_d = nc.inline_tensor(np.eye(SL, dtype=np.float32), name="ident")

    wav_ch = wav[:, :].rearrange("r (c l) -> (r c) l", c=NCH)   # [120, 32000]
    out_ch = out[:, :].rearrange("r (c l) -> (r c) l", c=NCH)

    n_full = (NSL // S) if (NSL % S) else (NSL // S - 1)
    iters = []
    done = 0
    while done < NSL:
        s = min(S, NSL - done)
        iters.append((done, s))
        done += s

    with tile.TileContext(nc) as tc:
        with (
            tc.tile_pool(name="const", bufs=1) as cpool,
            tc.tile_pool(name="io", bufs=3) as iopool,
            tc.tile_pool(name="work", bufs=3) as wpool,
            tc.tile_pool(name="psum", bufs=2, space="PSUM") as ppool,
        ):
            hA = cpool.tile([SL, SL], f32)
            nc.sync.dma_start(hA[:].bitcast(MM_DT), hA_d[:, :].bitcast(MM_DT))
            hB = cpool.tile([SL, SL], f32)
            nc.sync.dma_start(hB[:].bitcast(MM_DT), hB_d[:, :].bitcast(MM_DT))
            ident = cpool.tile([SL, SL], f32)
            nc.sync.dma_start(ident[:], id_d[:, :])

            # initial carry: the 128 samples preceding each chunk (zeros for
            # row-initial chunks), in X_nat layout then transposed.
            c0 = cpool.tile([P, SL], f32)
            nc.gpsimd.memset(c0[:], 0.0)
            for r in range(R):
                nc.sync.dma_start(
                    c0[r * NCH + 1: r * NCH + NCH, :],
                    wav_ch[r * NCH: r * NCH + NCH - 1, L - SL: L],
                )
            pc0 = ppool.tile([SL, P], f32, tag="pt")
            nc.tensor.transpose(
                pc0[:].bitcast(TP_DT), c0[:].bitcast(TP_DT),
                ident[:P, :P].bitcast(TP_DT),
            )

            prev_slab = None
            prev_s = None
            for (sl0, s) in iters:
                base = sl0 * SL
                w = s * SL

                xin = iopool.tile([P, S * SL], f32, tag="xin")
                nc.sync.dma_start(xin[:, :w], wav_ch[:, base: base + w])

                # transpose s slices of [P, 128] -> [128, P] into one PSUM bank
                pt = ppool.tile([SL, S * P], f32, tag="pt")
                for j in range(s):
                    nc.tensor.transpose(
                        pt[:, j * P: (j + 1) * P].bitcast(TP_DT),
                        xin[:, j * SL: (j + 1) * SL].bitcast(TP_DT),
                        ident[:P, :P].bitcast(TP_DT),
                    )

                # slab: slot 0 = carry (transposed previous slice), 1..s = new
                slab = wpool.tile([SL, (S + 1) * P], f32, tag="slab")
                nc.scalar.copy(slab[:, P: P + s * P].bitcast(MM_DT), pt[:, : s * P])
                if prev_slab is None:
                    nc.scalar.copy(slab[:, 0: P].bitcast(MM_DT), pc0[:, :])
                else:
                    nc.vector.tensor_copy(
                        slab[:, 0: P].bitcast(MM_DT),
                        prev_slab[:, prev_s * P: (prev_s + 1) * P],
                    )

                # banded FIR: y_T = HB^T @ slab[1..s] + HA^T @ slab[0..s-1]
                py = ppool.tile([SL, S * P], f32, tag="py")
                nc.tensor.matmul(
                    py[:, : s * P],
                    hB[:].bitcast(MM_DT),
                    slab[:, P: P + s * P].bitcast(MM_DT),
                    start=True, stop=False,
                )
                nc.tensor.matmul(
                    py[:, : s * P],
                    hA[:].bitcast(MM_DT),
                    slab[:, 0: s * P].bitcast(MM_DT),
                    start=False, stop=True,
                )

                ysb = wpool.tile([SL, S * P], f32, tag="ysb")
                nc.vector.tensor_copy(ysb[:, : s * P], py[:, : s * P])

                # transpose back: [128, P] slices -> [P, 128]
                po = ppool.tile([P, S * SL], f32, tag="po")
                for j in range(s):
                    nc.tensor.transpose(
                        po[:, j * SL: (j + 1) * SL].bitcast(TP_DT),
                        ysb[:, j * P: (j + 1) * P].bitcast(TP_DT),
                        ident[:, :].bitcast(TP_DT),
                    )

                yout = iopool.tile([P, S * SL], f32, tag="yout")
                nc.scalar.copy(yout[:, :w], po[:, :w])
                nc.sync.dma_start(out_ch[:, base: base + w], yout[:, :w])

                prev_slab = slab
                prev_s = s

    nc.finalize()
    return nc


_NC_CACHE = None


def _get_nc():
    global _NC_CACHE
    if _NC_CACHE is None:
        _NC_CACHE = _build()
    return _NC_CACHE


def _run(wav_full: np.ndarray, trace: bool = False):
    nc = _get_nc()
    wav_full = np.ascontiguousarray(wav_full, dtype=np.float32)
    in_maps = [
        {"wav": wav_full[i * R: (i + 1) * R]} for i in range(N_CORES)
    ]
    res = run_bass_kernel_spmd(
        nc, in_maps, core_ids=list(range(N_CORES)), trace=trace
    )
    out = np.concatenate([res.results[i]["out"] for i in range(N_CORES)], axis=0)
    return out, res


def kernel(wav: np.ndarray) -> np.ndarray:
    out, _ = _run(np.asarray(wav))
    return out


# revision 4
# speedup vs baseline: 1.2793x; 1.0809x over previous
"""Biquad lowpass filter (torchaudio lowpass_biquad, SR=24000, cutoff=8000, Q=0.707)
over wav [64, 480000], data-parallel across 8 TRN2 NeuronCores.

The biquad's poles have |z| = sqrt(a2) ~= 0.49, so its impulse response decays
below float32 resolution within ~48 samples. The IIR is therefore numerically
identical to a 64-tap causal FIR, which we evaluate on the TensorEngine:

  y[n*128 + i] = sum_k x[n*128 + k] * HB[k, i]  +  sum_k x[(n-1)*128 + k] * HA[k, i]
  HB[k, i] = h[i - k]        (0 <= i-k < D)
  HA[k, i] = h[128 + i - k]  (0 < 128+i-k < D)

Per core: 8 rows x 480000 samples, viewed as 120 chunks (8 rows x 15 chunks)
of 32000 samples. Time slices of 128 are PE-transposed so that within-slice
time sits on partitions, the two banded matmuls run in float32r, and the
result slices are PE-transposed back and streamed out contiguously.
"""

import sys

sys.path.insert(0, "/opt/trn_rl_repo")

import numpy as np

import concourse.bass as bass
import concourse.mybir as mybir
import concourse.tile as tile
from concourse import bacc
from concourse.bass_utils import run_bass_kernel_spmd

f32 = mybir.dt.float32
f32r = mybir.dt.float32r

# ---- problem constants ----------------------------------------------------
SR = 24000
CUTOFF = 8000.0
Q = 0.707

B_FULL, T = 64, 480000
N_CORES = 8
R = B_FULL // N_CORES          # rows per core
NCH = 15                       # chunks per row
P = R * NCH                    # 120 partitions in use
L = T // NCH                   # 32000 samples per chunk
SL = 128                       # slice length (transpose tile)
NSL = L // SL                  # 250 slices per chunk
S = 4                          # slices per matmul block
G = 4                          # matmul blocks per DMA transfer
D = 64                         # FIR taps kept (h[48] ~ 1e-15 already)

# matmul dtype: float32r runs the PE at 1 cycle/column for free-dim >= 256
MM_DT = f32r
TP_DT = f32r                   # dtype for PE transposes


def _filter_mats():
    w0 = 2.0 * np.pi * CUTOFF / SR
    alpha = np.sin(w0) / (2.0 * Q)
    cos_w0 = np.cos(w0)
    b0 = (1.0 - cos_w0) / 2.0
    b1 = 1.0 - cos_w0
    b2 = b0
    a0 = 1.0 + alpha
    a1 = -2.0 * cos_w0
    a2 = 1.0 - alpha
    b0, b1, b2, a1, a2 = (np.float32(b0 / a0), np.float32(b1 / a0),
                          np.float32(b2 / a0), np.float32(a1 / a0),
                          np.float32(a2 / a0))
    # impulse response in float64 using the float32-rounded coefficients
    h = np.zeros(D, dtype=np.float64)
    x1 = x2 = y1 = y2 = 0.0
    for t in range(D):
        x = 1.0 if t == 0 else 0.0
        y = (float(b0) * x + float(b1) * x1 + float(b2) * x2
             - float(a1) * y1 - float(a2) * y2)
        h[t] = y
        x2, x1 = x1, x
        y2, y1 = y1, y
    HB = np.zeros((SL, SL), dtype=np.float32)
    HA = np.zeros((SL, SL), dtype=np.float32)
    for k in range(SL):
        for i in range(SL):
            d = i - k
            if 0 <= d < D:
                HB[k, i] = h[d]
            d2 = SL + i - k
            if 0 < d2 < D:
                HA[k, i] = h[d2]
    return HA, HB


def _build():
    HA_np, HB_np = _filter_mats()
    nc = bacc.Bacc("TRN2", target_bir_lowering=False)

    wav = nc.dram_tensor("wav", [R, T], f32, kind="ExternalInput")
    out = nc.dram_tensor("out", [R, T], f32, kind="ExternalOutput")
    hA_d = nc.inline_tensor(HA_np, name="hA")
    hB_d = nc.inline_tensor(HB_np, name="hB")
    id_d = nc.inline_tensor(np.eye(SL, dtype=np.float32), name="ident")

    wav_ch = wav[:, :].rearrange("r (c l) -> (r c) l", c=NCH)   # [120, 32000]
    out_ch = out[:, :].rearrange("r (c l) -> (r c) l", c=NCH)

    # sub-iterations of up to S slices, grouped G sub-iters per DMA
    subs = []
    done = 0
    while done < NSL:
        s = min(S, NSL - done)
        subs.append((done, s))
        done += s
    groups = [subs[i: i + G] for i in range(0, len(subs), G)]

    with tile.TileContext(nc) as tc:
        with (
            tc.tile_pool(name="const", bufs=1) as cpool,
            tc.tile_pool(name="io", bufs=3) as iopool,
            tc.tile_pool(name="work", bufs=3) as wpool,
            tc.tile_pool(name="psum", bufs=2, space="PSUM") as ppool,
        ):
            hA = cpool.tile([SL, SL], f32)
            nc.sync.dma_start(hA[:].bitcast(MM_DT), hA_d[:, :].bitcast(MM_DT))
            hB = cpool.tile([SL, SL], f32)
            nc.sync.dma_start(hB[:].bitcast(MM_DT), hB_d[:, :].bitcast(MM_DT))
            ident = cpool.tile([SL, SL], f32)
            nc.sync.dma_start(ident[:], id_d[:, :])

            # initial carry: the 128 samples preceding each chunk (zeros for
            # row-initial chunks), in X_nat layout then transposed.
            c0 = cpool.tile([P, SL], f32)
            nc.gpsimd.memset(c0[:], 0.0)
            for r in range(R):
                nc.sync.dma_start(
                    c0[r * NCH + 1: r * NCH + NCH, :],
                    wav_ch[r * NCH: r * NCH + NCH - 1, L - SL: L],
                )
            pc0 = ppool.tile([SL, P], f32, tag="pt")
            nc.tensor.transpose(pc0[:], c0[:], ident[:P, :P])

            prev_slab = None
            prev_s = None
            for grp in groups:
                gbase = grp[0][0] * SL
                gw = sum(s for (_, s) in grp) * SL

                xin = iopool.tile([P, G * S * SL], f32, tag="xin")
                nc.sync.dma_start(
                    xin[:, :gw].bitcast(TP_DT),
                    wav_ch[:, gbase: gbase + gw].bitcast(TP_DT),
                )
                yout = iopool.tile([P, G * S * SL], f32, tag="yout")

                for (sl0, s) in grp:
                    off = sl0 * SL - gbase      # sample offset within group
                    w = s * SL

                    # transpose s slices of [P, 128] -> [128, P] into PSUM
                    pt = ppool.tile([SL, S * P], f32, tag="pt")
                    for j in range(s):
                        nc.tensor.transpose(
                            pt[:, j * P: (j + 1) * P].bitcast(TP_DT),
                            xin[:, off + j * SL: off + (j + 1) * SL].bitcast(TP_DT),
                            ident[:P, :P].bitcast(TP_DT),
                        )

                    # slab: slot 0 = carry (previous transposed slice), 1..s new
                    slab = wpool.tile([SL, (S + 1) * P], f32, tag="slab")
                    nc.scalar.copy(
                        slab[:, P: P + s * P].bitcast(MM_DT), pt[:, : s * P]
                    )
                    if prev_slab is None:
                        nc.scalar.copy(slab[:, 0: P].bitcast(MM_DT), pc0[:, :])
                    else:
                        nc.vector.tensor_copy(
                            slab[:, 0: P].bitcast(MM_DT),
                            prev_slab[:, prev_s * P: (prev_s + 1) * P],
                        )

                    # banded FIR: y_T = HB^T @ slab[1..s] + HA^T @ slab[0..s-1]
                    py = ppool.tile([SL, S * P], f32, tag="py")
                    nc.tensor.matmul(
                        py[:, : s * P],
                        hB[:].bitcast(MM_DT),
                        slab[:, P: P + s * P].bitcast(MM_DT),
                        start=True, stop=False,
                    )
                    nc.tensor.matmul(
                        py[:, : s * P],
                        hA[:].bitcast(MM_DT),
                        slab[:, 0: s * P].bitcast(MM_DT),
                        start=False, stop=True,
                    )

                    ysb = wpool.tile([SL, S * P], f32, tag="ysb")
                    nc.vector.tensor_copy(
                        ysb[:, : s * P].bitcast(TP_DT), py[:, : s * P]
                    )

                    # transpose back: [128, P] slices -> [P, 128]
                    po = ppool.tile([P, S * SL], f32, tag="po")
                    for j in range(s):
                        nc.tensor.transpose(
                            po[:, j * SL: (j + 1) * SL].bitcast(TP_DT),
                            ysb[:, j * P: (j + 1) * P].bitcast(TP_DT),
                            ident[:, :].bitcast(TP_DT),
                        )

                    nc.scalar.copy(yout[:, off: off + w], po[:, :w])

                    prev_slab = slab
                    prev_s = s

                nc.sync.dma_start(out_ch[:, gbase: gbase + gw], yout[:, :gw])

    nc.finalize()
    return nc


_NC_CACHE = None


def _get_nc():
    global _NC_CACHE
    if _NC_CACHE is None:
        _NC_CACHE = _build()
    return _NC_CACHE


def _run(wav_full: np.ndarray, trace: bool = False):
    nc = _get_nc()
    wav_full = np.ascontiguousarray(wav_full, dtype=np.float32)
    in_maps = [
        {"wav": wav_full[i * R: (i + 1) * R]} for i in range(N_CORES)
    ]
    res = run_bass_kernel_spmd(
        nc, in_maps, core_ids=list(range(N_CORES)), trace=trace
    )
    out = np.concatenate([res.results[i]["out"] for i in range(N_CORES)], axis=0)
    return out, res


def kernel(wav: np.ndarray) -> np.ndarray:
    out, _ = _run(np.asarray(wav))
    return out


# revision 5
# speedup vs baseline: 1.3561x; 1.0601x over previous
"""Biquad lowpass filter (torchaudio lowpass_biquad, SR=24000, cutoff=8000, Q=0.707)
over wav [64, 480000], data-parallel across 8 TRN2 NeuronCores.

The biquad's poles have |z| = sqrt(a2) ~= 0.49, so its impulse response decays
below float32 resolution within ~48 samples. The IIR is therefore numerically
identical to a 64-tap causal FIR, which we evaluate on the TensorEngine:

  y[n*128 + i] = sum_k x[n*128 + k] * HB[k, i]  +  sum_k x[(n-1)*128 + k] * HA[k, i]
  HB[k, i] = h[i - k]        (0 <= i-k < D)
  HA[k, i] = h[128 + i - k]  (0 < 128+i-k < D)

Per core: 8 rows x 480000 samples, viewed as 120 chunks (8 rows x 15 chunks)
of 32000 samples. Time slices of 128 are PE-transposed so that within-slice
time sits on partitions, the two banded matmuls run in float32r, and the
result slices are PE-transposed back and streamed out contiguously.
"""

import sys

sys.path.insert(0, "/opt/trn_rl_repo")

import numpy as np

import concourse.bass as bass
import concourse.mybir as mybir
import concourse.tile as tile
from concourse import bacc
from concourse.bass_utils import run_bass_kernel_spmd

f32 = mybir.dt.float32
f32r = mybir.dt.float32r

# ---- problem constants ----------------------------------------------------
SR = 24000
CUTOFF = 8000.0
Q = 0.707

B_FULL, T = 64, 480000
N_CORES = 8
R = B_FULL // N_CORES          # rows per core
NCH = 15                       # chunks per row
P = R * NCH                    # 120 partitions in use
L = T // NCH                   # 32000 samples per chunk
SL = 128                       # slice length (transpose tile)
NSL = L // SL                  # 250 slices per chunk
S = 4                          # slices per matmul block
G = 4                          # matmul blocks per DMA transfer
D = 64                         # FIR taps kept (h[48] ~ 1e-15 already)

# matmul dtype: float32r runs the PE at 1 cycle/column for free-dim >= 256
MM_DT = f32r
TP_DT = f32r                   # dtype for PE transposes


def _filter_mats():
    w0 = 2.0 * np.pi * CUTOFF / SR
    alpha = np.sin(w0) / (2.0 * Q)
    cos_w0 = np.cos(w0)
    b0 = (1.0 - cos_w0) / 2.0
    b1 = 1.0 - cos_w0
    b2 = b0
    a0 = 1.0 + alpha
    a1 = -2.0 * cos_w0
    a2 = 1.0 - alpha
    b0, b1, b2, a1, a2 = (np.float32(b0 / a0), np.float32(b1 / a0),
                          np.float32(b2 / a0), np.float32(a1 / a0),
                          np.float32(a2 / a0))
    # impulse response in float64 using the float32-rounded coefficients
    h = np.zeros(D, dtype=np.float64)
    x1 = x2 = y1 = y2 = 0.0
    for t in range(D):
        x = 1.0 if t == 0 else 0.0
        y = (float(b0) * x + float(b1) * x1 + float(b2) * x2
             - float(a1) * y1 - float(a2) * y2)
        h[t] = y
        x2, x1 = x1, x
        y2, y1 = y1, y
    HB = np.zeros((SL, SL), dtype=np.float32)
    HA = np.zeros((SL, SL), dtype=np.float32)
    for k in range(SL):
        for i in range(SL):
            d = i - k
            if 0 <= d < D:
                HB[k, i] = h[d]
            d2 = SL + i - k
            if 0 < d2 < D:
                HA[k, i] = h[d2]
    return HA, HB


def _build():
    HA_np, HB_np = _filter_mats()
    nc = bacc.Bacc("TRN2", target_bir_lowering=False)

    wav = nc.dram_tensor("wav", [R, T], f32, kind="ExternalInput")
    out = nc.dram_tensor("out", [R, T], f32, kind="ExternalOutput")
    hA_d = nc.inline_tensor(HA_np, name="hA")
    hB_d = nc.inline_tensor(HB_np, name="hB")
    id_d = nc.inline_tensor(np.eye(SL, dtype=np.float32), name="ident")

    wav_ch = wav[:, :].rearrange("r (c l) -> (r c) l", c=NCH)   # [120, 32000]
    out_ch = out[:, :].rearrange("r (c l) -> (r c) l", c=NCH)

    # sub-iterations of up to S slices, grouped G sub-iters per DMA
    subs = []
    done = 0
    while done < NSL:
        s = min(S, NSL - done)
        subs.append((done, s))
        done += s
    groups = [subs[i: i + G] for i in range(0, len(subs), G)]

    with tile.TileContext(nc) as tc:
        with (
            tc.tile_pool(name="const", bufs=1) as cpool,
            tc.tile_pool(name="io", bufs=4) as iopool,
            tc.tile_pool(name="work", bufs=3) as wpool,
            tc.tile_pool(name="psum", bufs=2, space="PSUM") as ppool,
        ):
            hA = cpool.tile([SL, SL], f32)
            nc.sync.dma_start(hA[:].bitcast(MM_DT), hA_d[:, :].bitcast(MM_DT))
            hB = cpool.tile([SL, SL], f32)
            nc.sync.dma_start(hB[:].bitcast(MM_DT), hB_d[:, :].bitcast(MM_DT))
            ident = cpool.tile([SL, SL], f32)
            nc.sync.dma_start(ident[:], id_d[:, :])

            # initial carry: the 128 samples preceding each chunk (zeros for
            # row-initial chunks), in X_nat layout then transposed.
            c0 = cpool.tile([P, SL], f32)
            nc.gpsimd.memset(c0[:], 0.0)
            for r in range(R):
                nc.sync.dma_start(
                    c0[r * NCH + 1: r * NCH + NCH, :],
                    wav_ch[r * NCH: r * NCH + NCH - 1, L - SL: L],
                )
            pc0 = ppool.tile([SL, P], f32, tag="pt")
            nc.tensor.transpose(pc0[:], c0[:], ident[:P, :P])

            prev_slab = None
            prev_s = None
            for gi, grp in enumerate(groups):
                gbase = grp[0][0] * SL
                gw = sum(s for (_, s) in grp) * SL

                xin = iopool.tile([P, G * S * SL], f32, tag="xin")
                in_eng = nc.sync if gi % 2 == 0 else nc.scalar
                in_eng.dma_start(
                    xin[:, :gw].bitcast(TP_DT),
                    wav_ch[:, gbase: gbase + gw].bitcast(TP_DT),
                )
                yout = iopool.tile([P, G * S * SL], f32, tag="yout")

                for (sl0, s) in grp:
                    off = sl0 * SL - gbase      # sample offset within group
                    w = s * SL

                    # transpose s slices of [P, 128] -> [128, P] into PSUM
                    pt = ppool.tile([SL, S * P], f32, tag="pt")
                    for j in range(s):
                        nc.tensor.transpose(
                            pt[:, j * P: (j + 1) * P].bitcast(TP_DT),
                            xin[:, off + j * SL: off + (j + 1) * SL].bitcast(TP_DT),
                            ident[:P, :P].bitcast(TP_DT),
                        )

                    # slab: slot 0 = carry (previous transposed slice), 1..s new
                    slab = wpool.tile([SL, (S + 1) * P], f32, tag="slab")
                    nc.scalar.copy(
                        slab[:, P: P + s * P].bitcast(MM_DT), pt[:, : s * P]
                    )
                    if prev_slab is None:
                        nc.scalar.copy(slab[:, 0: P].bitcast(MM_DT), pc0[:, :])
                    else:
                        nc.vector.tensor_copy(
                            slab[:, 0: P].bitcast(MM_DT),
                            prev_slab[:, prev_s * P: (prev_s + 1) * P],
                        )

                    # banded FIR: y_T = HB^T @ slab[1..s] + HA^T @ slab[0..s-1]
                    py = ppool.tile([SL, S * P], f32, tag="py")
                    nc.tensor.matmul(
                        py[:, : s * P],
                        hB[:].bitcast(MM_DT),
                        slab[:, P: P + s * P].bitcast(MM_DT),
                        start=True, stop=False,
                    )
                    nc.tensor.matmul(
                        py[:, : s * P],
                        hA[:].bitcast(MM_DT),
                        slab[:, 0: s * P].bitcast(MM_DT),
                        start=False, stop=True,
                    )

                    ysb = wpool.tile([SL, S * P], f32, tag="ysb")
                    nc.vector.tensor_copy(
                        ysb[:, : s * P].bitcast(TP_DT), py[:, : s * P]
                    )

                    # transpose back: [128, P] slices -> [P, 128]
                    po = ppool.tile([P, S * SL], f32, tag="po")
                    for j in range(s):
                        nc.tensor.transpose(
                            po[:, j * SL: (j + 1) * SL].bitcast(TP_DT),
                            ysb[:, j * P: (j + 1) * P].bitcast(TP_DT),
                            ident[:, :].bitcast(TP_DT),
                        )

                    nc.scalar.copy(yout[:, off: off + w], po[:, :w])

                    prev_slab = slab
                    prev_s = s

                out_eng = nc.scalar if gi % 2 == 0 else nc.sync
                out_eng.dma_start(out_ch[:, gbase: gbase + gw], yout[:, :gw])

    nc.finalize()
    return nc


_NC_CACHE = None


def _get_nc():
    global _NC_CACHE
    if _NC_CACHE is None:
        _NC_CACHE = _build()
    return _NC_CACHE


def _run(wav_full: np.ndarray, trace: bool = False):
    nc = _get_nc()
    wav_full = np.ascontiguousarray(wav_full, dtype=np.float32)
    in_maps = [
        {"wav": wav_full[i * R: (i + 1) * R]} for i in range(N_CORES)
    ]
    res = run_bass_kernel_spmd(
        nc, in_maps, core_ids=list(range(N_CORES)), trace=trace
    )
    out = np.concatenate([res.results[i]["out"] for i in range(N_CORES)], axis=0)
    return out, res


def kernel(wav: np.ndarray) -> np.ndarray:
    out, _ = _run(np.asarray(wav))
    return out
